# revision 9
# baseline (speedup 1.0000x reference)
"""Trainium2 Bass kernel for a GNN message-passing layer.

reference semantics (jax):
    src, dst = edge_index
    messages   = silu(concat(nodes[src], edge_features) @ mw1 + mb1)    # [E, D]
    aggregated = segment_sum(messages, dst, N)                          # [N, D]
    updated    = silu(concat(nodes, aggregated) @ uw1 + ub1) @ uw2 + ub2
    out        = nodes + updated

Distribution: destination-node partition across 8 cores. Each core owns a
contiguous 1/8 slice of the (padded) node range, aggregates exactly the
edges landing in its slice, and runs the update MLP on its slice. No
collectives.

Host-side work is limited to layout transforms of inputs (slicing,
padding, permutation/gather of input rows into slot order, per-tile
128x128 block transposes, bf16 byte-truncation, index tables) — no float
arithmetic.

Slot layout: edges are bucketed by destination node tile (128 dst nodes
per tile). Local tile t owns kt[t] edge tiles of 128 slots (kt = max
over cores, a compile-time constant); leftover slots are pads with
dst-offset -1 so their junk messages scatter with weight 0. The host
streams, per edge slot, BOTH the source-node row nodes[src] and the
edge-feature row (pre-transposed per 128-tile, bf16), so the device does
no gathers at all.

Device pipeline per core, per local node tile t:
  1. One contiguous DMA of the [ns^T | ef^T] chunk (bf16, [128, 2*kt*128]).
  2. Per 4-edge-tile chunk: per edge tile a 1-partition ones-matmul adds
     the message bias into PSUM (start=True), then ns/ef matmuls
     accumulate; one SiLU (PSUM -> SBUF bf16); one wide DVE is_equal
     builds all 4 one-hots at once (broadcast dst-offset columns vs a
     tiled iota).
  3. Per edge tile: a scatter matmul (lhsT=msg, rhs=one-hot)
     accumulating agg^T [d, j] in PSUM.
  Chunks are software-pipelined: chunk i's matmuls are emitted before
  chunk i-1's silu/scatter consumers so the PE stream never waits on the
  ACT/DVE roundtrip.
  4. Update MLP in transposed space (4 node tiles per group), residual,
     transpose back, store (partition-major output, host re-layouts).
"""

import math
import sys

sys.path.insert(0, "/opt/trn_rl_repo")

import numpy as np
import ml_dtypes

import concourse.bacc as bacc
import concourse.mybir as mybir
import concourse.tile as tile
from concourse import bass_utils

P = 128
C = 8  # cores
ONEHOT_GPSIMD = False  # Pool engine fails ISA check for is_equal tensor_tensor

F32 = mybir.dt.float32
BF16 = mybir.dt.bfloat16
AF = mybir.ActivationFunctionType
OP = mybir.AluOpType

NP_BF16 = ml_dtypes.bfloat16


def _trunc_bf16(a):
    """fp32 -> bf16 by byte truncation (pure byte slicing, no arithmetic)."""
    a = np.ascontiguousarray(a, np.float32)
    return a.view(np.uint16)[..., 1::2].view(NP_BF16)


def _blocksT(a):
    """[B*P, D] -> [P, B*D]: per-128-row-block transpose, blocks along free dim.

    out[d, b*D + e ... ] wait: out[x, b*P + e] = a[b*P + e, x]; requires D == P.
    """
    B = a.shape[0] // P
    D = a.shape[1]
    # [B, P, D] -> [B, D, P] -> [D?, ...] place block b at cols [b*P, (b+1)*P)
    t = a.reshape(B, P, D).transpose(2, 0, 1)  # [D, B, P]
    return np.ascontiguousarray(t.reshape(D, B * P))


def _host_prep(nodes, edge_index, edge_features, ntiles_pc):
    """Bucket edges by destination node tile; build per-core slot streams."""
    N, D = nodes.shape
    E = edge_index.shape[1]
    ntiles = ntiles_pc * C

    src = edge_index[0].astype(np.int64)
    dst = edge_index[1].astype(np.int64)
    tileid = dst // P
    order = np.argsort(tileid, kind="stable")
    ds = dst[order]
    ss = src[order]
    tid_s = tileid[order]

    counts = np.bincount(tileid, minlength=ntiles)
    cpt = counts.reshape(C, ntiles_pc)
    kt = [max(1, int(math.ceil(cpt[:, t].max() / P))) for t in range(ntiles_pc)]
    offs = np.zeros(ntiles_pc + 1, np.int64)
    np.cumsum(kt, out=offs[1:])
    sumkt = int(offs[-1])
    SL = sumkt * P  # slots per core

    tile_start = np.zeros(ntiles + 1, np.int64)
    np.cumsum(counts, out=tile_start[1:])
    rank = np.arange(E, dtype=np.int64) - tile_start[tid_s]
    core = tid_s // ntiles_pc
    t_local = tid_s % ntiles_pc
    slot = offs[t_local] * P + rank  # slot within the core's stream

    nodes16 = _trunc_bf16(nodes)
    ef16 = _trunc_bf16(edge_features)

    per_core = []
    for c in range(C):
        m = core == c
        sl_c = slot[m]
        # source rows + edge rows into slot order (pads stay zero)
        ns = np.zeros((SL, D), NP_BF16)
        ns[sl_c] = nodes16[ss[m]]
        ef = np.zeros((SL, D), NP_BF16)
        ef[sl_c] = ef16[order[m]]
        dof = np.full(SL, -1.0, np.float32)
        dof[sl_c] = (ds[m] - (ds[m] // P) * P).astype(np.float32)

        nsT = _blocksT(ns)  # [P, SL]
        efT = _blocksT(ef)  # [P, SL]
        # merged stream: per tile t, kt[t] ns-tiles then kt[t] ef-tiles
        nsef = np.empty((P, 2 * SL), NP_BF16)
        for t in range(ntiles_pc):
            a, b = int(offs[t]) * P, int(offs[t + 1]) * P
            w = b - a
            nsef[:, 2 * a : 2 * a + w] = nsT[:, a:b]
            nsef[:, 2 * a + w : 2 * b] = efT[:, a:b]
        dstoffT = np.ascontiguousarray(
            _trunc_bf16(dof.reshape(sumkt, P).T)
        )  # [P, sumkt]
        per_core.append(dict(nsefT=nsef, dstoffT=dstoffT))
    return kt, per_core


def build_program(D, ntiles_pc, kt, debug=False):
    """Build the SPMD Bass program (identical across cores)."""
    assert D == P
    nc = bacc.Bacc("TRN2", target_bir_lowering=False, debug=False, num_devices=C)
    NP_ = ntiles_pc * P
    offs = np.zeros(ntiles_pc + 1, np.int64)
    np.cumsum(kt, out=offs[1:])
    sumkt = int(offs[-1])
    ktmax = max(kt)

    d = lambda name, shape, dt=F32, kind="ExternalInput": nc.dram_tensor(
        name, shape, dt, kind=kind
    ).ap()

    nsef = d("nsefT", [P, 2 * sumkt * P], BF16)
    doff = d("dstoffT", [P, sumkt], BF16)
    ownT_d = d("own_nodesT", [P, NP_])
    wt = d("wt", [D, D], BF16)
    wb_ = d("wb", [D, D], BF16)
    mb1r = d("mb1r", [1, D], BF16)
    ones1 = d("ones1", [1, P], BF16)
    iota4 = d("iota4", [P, 4 * P], BF16)
    ua = d("ua", [D, D])
    ub = d("ub", [D, D])
    uw2 = d("uw2", [D, D])
    ub1c = d("ub1c", [P, 1])
    ub2c = d("ub2c", [P, 1])
    ident = d("ident", [P, P])
    out = d("out_own", [P, NP_], kind="ExternalOutput")
    aggdbg = d("aggdbg", [P, ntiles_pc * D], kind="ExternalOutput") if debug else None

    with tile.TileContext(nc) as tc:
        with (
            tc.tile_pool(name="const", bufs=1) as cp,
            tc.tile_pool(name="sb", bufs=3) as sb,
            tc.tile_pool(name="big", bufs=3) as bigp,
            tc.tile_pool(name="psum", bufs=2, space="PSUM") as pp,
            tc.tile_pool(name="psum1", bufs=1, space="PSUM") as pp1,
            tc.tile_pool(name="psum3", bufs=3, space="PSUM") as pp3,
        ):
            def load_const(ap, shape, dt=F32):
                t = cp.tile(shape, dt, tag=ap.name)
                nc.sync.dma_start(out=t[:], in_=ap[:])
                return t

            wt_s = load_const(wt, [D, D], BF16)
            wb_s = load_const(wb_, [D, D], BF16)
            mb1_s = load_const(mb1r, [1, D], BF16)
            ones_s = load_const(ones1, [1, P], BF16)
            iota4_s = load_const(iota4, [P, 4 * P], BF16)
            ua_s = load_const(ua, [D, D])
            ub_s = load_const(ub, [D, D])
            uw2_s = load_const(uw2, [D, D])
            ub1_s = load_const(ub1c, [P, 1])
            ub2_s = load_const(ub2c, [P, 1])
            id_s = load_const(ident, [P, P])
            doff_s = load_const(doff, [P, sumkt], BF16)
            aggT_all = cp.tile([P, ntiles_pc * D], F32, tag="aggT_all")

            # ---- stage 2: edge pipeline (software-pipelined by 1 chunk) ----
            chunks = []
            for t in range(ntiles_pc):
                nch = math.ceil(kt[t] / 4)
                for ci in range(nch):
                    chunks.append((t, ci, ci == 0, ci == nch - 1))

            state = {}  # t -> (chunk_tile, paggT)

            def produce(t, ci, first):
                KT = kt[t]
                if first:
                    W2 = 2 * KT * D
                    ctile = bigp.tile([P, 2 * ktmax * D], BF16, tag="chunk")
                    dma = nc.sync if (t % 2 == 0) else nc.scalar
                    dma.dma_start(
                        out=ctile[:, :W2],
                        in_=nsef[
                            :, 2 * int(offs[t]) * D : 2 * int(offs[t]) * D + W2
                        ],
                    )
                    paggT = pp1.tile([P, D], F32, tag="paggT")
                    state[t] = (ctile, paggT)
                ctile, _ = state[t]
                k0 = ci * 4
                cw = min(4, KT - k0)
                pmsg = pp3.tile([P, 4 * P], F32, tag="pmsg")
                for j in range(cw):
                    k = k0 + j
                    reg = pmsg[:, j * P : (j + 1) * P]
                    nc.tensor.matmul(
                        out=reg, lhsT=ones_s[:], rhs=mb1_s[:],
                        start=True, stop=False,
                    )
                    nc.tensor.matmul(
                        out=reg, lhsT=ctile[:, k * D : (k + 1) * D], rhs=wt_s[:],
                        start=False, stop=False,
                    )
                    nc.tensor.matmul(
                        out=reg,
                        lhsT=ctile[:, (KT + k) * D : (KT + k + 1) * D],
                        rhs=wb_s[:],
                        start=False, stop=True,
                    )
                oh4 = sb.tile([P, 4 * P], BF16, tag="oh4")
                a = int(offs[t]) + k0
                nc.vector.tensor_tensor(
                    out=oh4[:, : cw * P].rearrange("p (f e) -> p f e", e=P),
                    in0=doff_s[:, a : a + cw].to_broadcast([P, cw, P]),
                    in1=iota4_s[:, : cw * P].rearrange("p (f e) -> p f e", e=P),
                    op=OP.is_equal,
                )
                return (t, ci, cw, pmsg, oh4)

            def consume(pend, last):
                t, ci, cw, pmsg, oh4 = pend
                KT = kt[t]
                k0 = ci * 4
                W = cw * P
                _, paggT = state[t]
                msg = sb.tile([P, 4 * P], BF16, tag="msg")
                nc.scalar.activation(out=msg[:, :W], in_=pmsg[:, :W], func=AF.Silu)
                for j in range(cw):
                    k = k0 + j
                    # aggT[d, j] += sum_e msg[e, d] * oh[e, j]
                    nc.tensor.matmul(
                        out=paggT[:],
                        lhsT=msg[:, j * P : (j + 1) * P],
                        rhs=oh4[:, j * P : (j + 1) * P],
                        start=(k == 0),
                        stop=(k == KT - 1),
                    )
                if last:
                    nc.vector.tensor_copy(
                        out=aggT_all[:, t * D : (t + 1) * D], in_=paggT[:]
                    )

            pend = None
            for t, ci, first, last in chunks:
                cur = produce(t, ci, first)
                if pend is not None:
                    consume(pend, pend_last)
                pend, pend_last = cur, last
            consume(pend, pend_last)
            if debug:
                nc.sync.dma_start(out=aggdbg[:], in_=aggT_all[:])

            # ---- stage 3: update MLP (transposed space, 4 node tiles/group) ----
            for g in range(0, ntiles_pc, 4):
                gw = min(4, ntiles_pc - g)
                W = gw * P
                ownT = sb.tile([P, 4 * P], F32, tag="ownT")
                nc.sync.dma_start(
                    out=ownT[:, :W], in_=ownT_d[:, g * P : g * P + W]
                )
                ph = pp.tile([P, 4 * P], F32, tag="ph")
                nc.tensor.matmul(
                    out=ph[:, :W], lhsT=ua_s[:], rhs=ownT[:, :W], start=True, stop=False
                )
                nc.tensor.matmul(
                    out=ph[:, :W],
                    lhsT=ub_s[:],
                    rhs=aggT_all[:, g * D : g * D + W],
                    start=False,
                    stop=True,
                )
                hT = sb.tile([P, 4 * P], F32, tag="hT")
                nc.scalar.activation(
                    out=hT[:, :W], in_=ph[:, :W], func=AF.Silu, bias=ub1_s[:, :1]
                )
                po = pp.tile([P, 4 * P], F32, tag="ph")
                nc.tensor.matmul(
                    out=po[:, :W], lhsT=uw2_s[:], rhs=hT[:, :W], start=True, stop=True
                )
                oT = sb.tile([P, 4 * P], F32, tag="oT")
                nc.scalar.activation(
                    out=oT[:, :W], in_=po[:, :W], func=AF.Identity, bias=ub2_s[:, :1]
                )
                nc.vector.tensor_tensor(
                    out=oT[:, :W], in0=oT[:, :W], in1=ownT[:, :W], op=OP.add
                )
                pOut = pp.tile([P, 4 * P], F32, tag="ptr")
                for j in range(gw):
                    nc.tensor.transpose(
                        out=pOut[:, j * P : (j + 1) * P],
                        in_=oT[:, j * P : (j + 1) * P],
                        identity=id_s[:],
                    )
                ot = sb.tile([P, 4 * P], F32, tag="ot")
                nc.vector.tensor_copy(out=ot[:, :W], in_=pOut[:, :W])
                nc.sync.dma_start(
                    out=out[:, g * P : g * P + W], in_=ot[:, :W]
                )

    nc.compile()
    return nc


def _run(nc, in_maps, trace=False):
    return bass_utils.run_bass_kernel_spmd(
        nc, in_maps, core_ids=list(range(C)), trace=trace
    )


def make_in_maps(nodes, edge_index, edge_features, mw1, mb1, uw1, ub1, uw2, ub2,
                 ntiles_pc):
    N, D = nodes.shape
    NP_ = ntiles_pc * P
    N2 = NP_ * C
    kt, per_core = _host_prep(nodes, edge_index, edge_features, ntiles_pc)

    nodes_pad = np.zeros((N2, D), np.float32)
    nodes_pad[:N] = nodes
    iota4 = np.broadcast_to(
        np.tile(np.arange(P, dtype=np.float32), 4), (P, 4 * P)
    )
    ident = np.eye(P, dtype=np.float32)

    shared = dict(
        wt=_trunc_bf16(mw1[:D]),
        wb=_trunc_bf16(mw1[D:]),
        mb1r=_trunc_bf16(mb1.reshape(1, D)),
        ones1=np.ones((1, P), NP_BF16),
        iota4=_trunc_bf16(np.ascontiguousarray(iota4)),
        ua=np.ascontiguousarray(uw1[:D], np.float32),
        ub=np.ascontiguousarray(uw1[D:], np.float32),
        uw2=np.ascontiguousarray(uw2, np.float32),
        ub1c=np.ascontiguousarray(ub1.reshape(D, 1), np.float32),
        ub2c=np.ascontiguousarray(ub2.reshape(D, 1), np.float32),
        ident=ident,
    )
    in_maps = []
    for c in range(C):
        m = dict(shared)
        own = nodes_pad[c * NP_ : (c + 1) * NP_]  # [NP_, D]
        m["own_nodesT"] = np.ascontiguousarray(own.T)  # [P(d), NP_]
        m["nsefT"] = per_core[c]["nsefT"]
        m["dstoffT"] = per_core[c]["dstoffT"]
        in_maps.append(m)
    return kt, in_maps


def kernel(nodes, edge_index, edge_features, mw1, mb1, uw1, ub1, uw2, ub2):
    nodes = np.asarray(nodes, np.float32)
    edge_index = np.asarray(edge_index, np.int32)
    edge_features = np.asarray(edge_features, np.float32)
    N, D = nodes.shape
    ntiles_pc = math.ceil(N / (C * P))
    kt, in_maps = make_in_maps(
        nodes, edge_index, edge_features, mw1, mb1, uw1, ub1, uw2, ub2, ntiles_pc
    )
    nc = build_program(D, ntiles_pc, kt)
    res = _run(nc, in_maps)
    NP_ = ntiles_pc * P
    # out_own is [P(d? no: partition = n%128), ntiles*D] -> rows
    outs = []
    for c in range(C):
        o = res.results[c]["out_own"]  # [P, NP_] with o[p, t*D+d] = row(t*128+p, d)
        outs.append(
            o.reshape(P, ntiles_pc, D).transpose(1, 0, 2).reshape(NP_, D)
        )
    out = np.concatenate(outs, axis=0)
    return out[:N].astype(np.float32)


if __name__ == "__main__":
    rng = np.random.default_rng(0)
    N, E, D = 4096, 16384, 128
    nodes = rng.standard_normal((N, D), dtype=np.float32)
    edge_index = rng.integers(0, N, (2, E)).astype(np.int32)
    ef = rng.standard_normal((E, D), dtype=np.float32)
    s2, s1 = 1 / np.sqrt(2 * D), 1 / np.sqrt(D)
    mw1 = rng.uniform(-s2, s2, (2 * D, D)).astype(np.float32)
    mb1 = rng.uniform(-s2, s2, D).astype(np.float32)
    uw1 = rng.uniform(-s2, s2, (2 * D, D)).astype(np.float32)
    ub1 = rng.uniform(-s2, s2, D).astype(np.float32)
    uw2 = rng.uniform(-s1, s1, (D, D)).astype(np.float32)
    ub2 = rng.uniform(-s1, s1, D).astype(np.float32)

    def silu(x):
        return x / (1 + np.exp(-x))

    def ref():
        src, dst = edge_index
        msg = silu(np.concatenate([nodes[src], ef], 1) @ mw1 + mb1)
        agg = np.zeros((N, D), np.float32)
        np.add.at(agg, dst, msg)
        upd = silu(np.concatenate([nodes, agg], 1) @ uw1 + ub1) @ uw2 + ub2
        return nodes + upd
    out = kernel(nodes, edge_index, ef, mw1, mb1, uw1, ub1, uw2, ub2)
    exp = ref()
    err = np.abs(out - exp).max() / np.abs(exp).max()
    print("tiny rel err:", err)


# revision 10
# speedup vs baseline: 1.5623x; 1.5623x over previous
"""Trainium2 Bass kernel for a GNN message-passing layer.

reference semantics (jax):
    src, dst = edge_index
    messages   = silu(concat(nodes[src], edge_features) @ mw1 + mb1)    # [E, D]
    aggregated = segment_sum(messages, dst, N)                          # [N, D]
    updated    = silu(concat(nodes, aggregated) @ uw1 + ub1) @ uw2 + ub2
    out        = nodes + updated

Distribution: destination-node partition across 8 cores. Each core owns a
contiguous 1/8 slice of the (padded) node range, aggregates exactly the
edges landing in its slice, and runs the update MLP on its slice. No
collectives.

Host-side work is limited to layout transforms of inputs (slicing,
padding, permutation/gather of input rows into slot order, per-tile
128x128 block transposes, bf16 byte-truncation, index tables) — no float
arithmetic.

Slot layout: edges are bucketed by destination node tile (128 dst nodes
per tile). Local tile t owns kt[t] edge tiles of 128 slots (kt = max
over cores, a compile-time constant); leftover slots are pads with
dst-offset -1 so their junk messages scatter with weight 0. The host
streams, per edge slot, BOTH the source-node row nodes[src] and the
edge-feature row (pre-transposed per 128-tile, bf16), so the device does
no gathers at all.

Device pipeline per core, per local node tile t:
  1. One contiguous DMA of the [ns^T | ef^T] chunk (bf16, [128, 2*kt*128]).
  2. Per 4-edge-tile chunk: per edge tile a 1-partition ones-matmul adds
     the message bias into PSUM (start=True), then ns/ef matmuls
     accumulate; one SiLU (PSUM -> SBUF bf16); one wide DVE is_equal
     builds all 4 one-hots at once (broadcast dst-offset columns vs a
     tiled iota).
  3. Per edge tile: a scatter matmul (lhsT=msg, rhs=one-hot)
     accumulating agg^T [d, j] in PSUM.
  Chunks are software-pipelined: chunk i's matmuls are emitted before
  chunk i-1's silu/scatter consumers so the PE stream never waits on the
  ACT/DVE roundtrip.
  4. Update MLP in transposed space (4 node tiles per group), residual,
     transpose back, store (partition-major output, host re-layouts).
"""

import math
import sys

sys.path.insert(0, "/opt/trn_rl_repo")

import numpy as np
import ml_dtypes

import concourse.bacc as bacc
import concourse.mybir as mybir
import concourse.tile as tile
from concourse import bass_utils

P = 128
C = 8  # cores
ONEHOT_GPSIMD = False  # Pool engine fails ISA check for is_equal tensor_tensor

F32 = mybir.dt.float32
BF16 = mybir.dt.bfloat16
AF = mybir.ActivationFunctionType
OP = mybir.AluOpType

NP_BF16 = ml_dtypes.bfloat16


def _trunc_bf16(a):
    """fp32 -> bf16 by byte truncation (pure byte slicing, no arithmetic)."""
    a = np.ascontiguousarray(a, np.float32)
    return a.view(np.uint16)[..., 1::2].view(NP_BF16)


def _blocksT(a):
    """[B*P, D] -> [P, B*D]: per-128-row-block transpose, blocks along free dim.

    out[d, b*D + e ... ] wait: out[x, b*P + e] = a[b*P + e, x]; requires D == P.
    """
    B = a.shape[0] // P
    D = a.shape[1]
    # [B, P, D] -> [B, D, P] -> [D?, ...] place block b at cols [b*P, (b+1)*P)
    t = a.reshape(B, P, D).transpose(2, 0, 1)  # [D, B, P]
    return np.ascontiguousarray(t.reshape(D, B * P))


def _host_prep(nodes, edge_index, edge_features, ntiles_pc):
    """Bucket edges by destination node tile; build per-core slot streams."""
    N, D = nodes.shape
    E = edge_index.shape[1]
    ntiles = ntiles_pc * C

    src = edge_index[0].astype(np.int64)
    dst = edge_index[1].astype(np.int64)
    tileid = dst // P
    order = np.argsort(tileid, kind="stable")
    ds = dst[order]
    ss = src[order]
    tid_s = tileid[order]

    counts = np.bincount(tileid, minlength=ntiles)
    cpt = counts.reshape(C, ntiles_pc)
    kt = [max(1, int(math.ceil(cpt[:, t].max() / P))) for t in range(ntiles_pc)]
    offs = np.zeros(ntiles_pc + 1, np.int64)
    np.cumsum(kt, out=offs[1:])
    sumkt = int(offs[-1])
    SL = sumkt * P  # slots per core

    tile_start = np.zeros(ntiles + 1, np.int64)
    np.cumsum(counts, out=tile_start[1:])
    rank = np.arange(E, dtype=np.int64) - tile_start[tid_s]
    core = tid_s // ntiles_pc
    t_local = tid_s % ntiles_pc
    slot = offs[t_local] * P + rank  # slot within the core's stream

    nodes16 = _trunc_bf16(nodes)
    ef16 = _trunc_bf16(edge_features)

    per_core = []
    for c in range(C):
        m = core == c
        sl_c = slot[m]
        # source rows + edge rows into slot order (pads stay zero)
        ns = np.zeros((SL, D), NP_BF16)
        ns[sl_c] = nodes16[ss[m]]
        ef = np.zeros((SL, D), NP_BF16)
        ef[sl_c] = ef16[order[m]]
        dof = np.full(SL, -1.0, np.float32)
        dof[sl_c] = (ds[m] - (ds[m] // P) * P).astype(np.float32)

        nsT = _blocksT(ns)  # [P, SL]
        efT = _blocksT(ef)  # [P, SL]
        # merged stream: per tile t, kt[t] ns-tiles then kt[t] ef-tiles
        nsef = np.empty((P, 2 * SL), NP_BF16)
        for t in range(ntiles_pc):
            a, b = int(offs[t]) * P, int(offs[t + 1]) * P
            w = b - a
            nsef[:, 2 * a : 2 * a + w] = nsT[:, a:b]
            nsef[:, 2 * a + w : 2 * b] = efT[:, a:b]
        dstoffT = np.ascontiguousarray(
            _trunc_bf16(dof.reshape(sumkt, P).T)
        )  # [P, sumkt]
        per_core.append(dict(nsefT=nsef, dstoffT=dstoffT))
    return kt, per_core


def build_program(D, ntiles_pc, kt, debug=False):
    """Build the SPMD Bass program (identical across cores)."""
    assert D == P
    nc = bacc.Bacc("TRN2", target_bir_lowering=False, debug=False, num_devices=C)
    NP_ = ntiles_pc * P
    offs = np.zeros(ntiles_pc + 1, np.int64)
    np.cumsum(kt, out=offs[1:])
    sumkt = int(offs[-1])
    ktmax = max(kt)

    d = lambda name, shape, dt=F32, kind="ExternalInput": nc.dram_tensor(
        name, shape, dt, kind=kind
    ).ap()

    nsef = d("nsefT", [P, 2 * sumkt * P], BF16)
    doff = d("dstoffT", [P, sumkt], BF16)
    ownT_d = d("own_nodesT", [P, NP_])
    wt = d("wt", [D, D], BF16)
    wb_ = d("wb", [D, D], BF16)
    mbB = d("mbB", [P, 4 * D])
    iota4 = d("iota4", [P, 4 * P], BF16)
    ua = d("ua", [D, D], BF16)
    ub = d("ub", [D, D], BF16)
    uw2 = d("uw2", [D, D], BF16)
    ub1c = d("ub1c", [P, 1])
    ub2c = d("ub2c", [P, 1])
    ident = d("ident", [P, P])
    out = d("out_own", [P, NP_], kind="ExternalOutput")
    aggdbg = d("aggdbg", [P, ntiles_pc * D], kind="ExternalOutput") if debug else None

    with tile.TileContext(nc) as tc:
        with (
            tc.tile_pool(name="const", bufs=1) as cp,
            tc.tile_pool(name="sb", bufs=3) as sb,
            tc.tile_pool(name="big", bufs=3) as bigp,
            tc.tile_pool(name="psum", bufs=2, space="PSUM") as pp,
            tc.tile_pool(name="psum1", bufs=1, space="PSUM") as pp1,
            tc.tile_pool(name="psum3", bufs=3, space="PSUM") as pp3,
        ):
            def load_const(ap, shape, dt=F32):
                t = cp.tile(shape, dt, tag=ap.name)
                nc.sync.dma_start(out=t[:], in_=ap[:])
                return t

            wt_s = load_const(wt, [D, D], BF16)
            wb_s = load_const(wb_, [D, D], BF16)
            mbB_s = load_const(mbB, [P, 4 * D])
            iota4_s = load_const(iota4, [P, 4 * P], BF16)
            ua_s = load_const(ua, [D, D], BF16)
            ub_s = load_const(ub, [D, D], BF16)
            uw2_s = load_const(uw2, [D, D], BF16)
            ub1_s = load_const(ub1c, [P, 1])
            ub2_s = load_const(ub2c, [P, 1])
            id_s = load_const(ident, [P, P])
            doff_s = load_const(doff, [P, sumkt], BF16)
            aggT_all = cp.tile([P, ntiles_pc * D], F32, tag="aggT_all")

            # ---- stage 2: edge pipeline (software-pipelined by 1 chunk) ----
            chunks = []
            for t in range(ntiles_pc):
                nch = math.ceil(kt[t] / 4)
                for ci in range(nch):
                    chunks.append((t, ci, ci == 0, ci == nch - 1))

            state = {}  # t -> (chunk_tile, paggT)

            def produce(t, ci, first):
                KT = kt[t]
                if first:
                    W2 = 2 * KT * D
                    ctile = bigp.tile([P, 2 * ktmax * D], BF16, tag="chunk")
                    dma = nc.sync if (t % 2 == 0) else nc.scalar
                    dma.dma_start(
                        out=ctile[:, :W2],
                        in_=nsef[
                            :, 2 * int(offs[t]) * D : 2 * int(offs[t]) * D + W2
                        ],
                    )
                    paggT = pp1.tile([P, D], F32, tag="paggT")
                    state[t] = (ctile, paggT)
                ctile, _ = state[t]
                k0 = ci * 4
                cw = min(4, KT - k0)
                pmsg = pp3.tile([P, 4 * P], F32, tag="pmsg")
                for j in range(cw):
                    k = k0 + j
                    reg = pmsg[:, j * P : (j + 1) * P]
                    nc.tensor.matmul(
                        out=reg, lhsT=ctile[:, k * D : (k + 1) * D], rhs=wt_s[:],
                        start=True, stop=False,
                    )
                    nc.tensor.matmul(
                        out=reg,
                        lhsT=ctile[:, (KT + k) * D : (KT + k + 1) * D],
                        rhs=wb_s[:],
                        start=False, stop=True,
                    )
                oh4 = sb.tile([P, 4 * P], BF16, tag="oh4")
                a = int(offs[t]) + k0
                nc.vector.tensor_tensor(
                    out=oh4[:, : cw * P].rearrange("p (f e) -> p f e", e=P),
                    in0=doff_s[:, a : a + cw].to_broadcast([P, cw, P]),
                    in1=iota4_s[:, : cw * P].rearrange("p (f e) -> p f e", e=P),
                    op=OP.is_equal,
                )
                return (t, ci, cw, pmsg, oh4)

            def consume(pend, last):
                t, ci, cw, pmsg, oh4 = pend
                KT = kt[t]
                k0 = ci * 4
                W = cw * P
                _, paggT = state[t]
                mpre = sb.tile([P, 4 * P], BF16, tag="mpre")
                nc.vector.tensor_tensor(
                    out=mpre[:, :W], in0=pmsg[:, :W], in1=mbB_s[:, :W], op=OP.add
                )
                msg = sb.tile([P, 4 * P], BF16, tag="msg")
                nc.scalar.activation(out=msg[:, :W], in_=mpre[:, :W], func=AF.Silu)
                for j in range(cw):
                    k = k0 + j
                    # aggT[d, j] += sum_e msg[e, d] * oh[e, j]
                    nc.tensor.matmul(
                        out=paggT[:],
                        lhsT=msg[:, j * P : (j + 1) * P],
                        rhs=oh4[:, j * P : (j + 1) * P],
                        start=(k == 0),
                        stop=(k == KT - 1),
                    )
                if last:
                    nc.vector.tensor_copy(
                        out=aggT_all[:, t * D : (t + 1) * D], in_=paggT[:]
                    )

            pend = None
            for t, ci, first, last in chunks:
                cur = produce(t, ci, first)
                if pend is not None:
                    consume(pend, pend_last)
                pend, pend_last = cur, last
            consume(pend, pend_last)
            if debug:
                nc.sync.dma_start(out=aggdbg[:], in_=aggT_all[:])

            # ---- stage 3: update MLP (transposed space, 4 node tiles/group;
            # output stays transposed [d, n] — host re-layouts) ----
            for g in range(0, ntiles_pc, 4):
                gw = min(4, ntiles_pc - g)
                W = gw * P
                ownT = sb.tile([P, 4 * P], F32, tag="ownT")
                nc.sync.dma_start(
                    out=ownT[:, :W], in_=ownT_d[:, g * P : g * P + W]
                )
                ownT16 = sb.tile([P, 4 * P], BF16, tag="ownT16")
                nc.vector.tensor_copy(out=ownT16[:, :W], in_=ownT[:, :W])
                agg16 = sb.tile([P, 4 * P], BF16, tag="agg16")
                nc.vector.tensor_copy(
                    out=agg16[:, :W], in_=aggT_all[:, g * D : g * D + W]
                )
                ph = pp.tile([P, 4 * P], F32, tag="ph")
                nc.tensor.matmul(
                    out=ph[:, :W], lhsT=ua_s[:], rhs=ownT16[:, :W],
                    start=True, stop=False,
                )
                nc.tensor.matmul(
                    out=ph[:, :W], lhsT=ub_s[:], rhs=agg16[:, :W],
                    start=False, stop=True,
                )
                hT = sb.tile([P, 4 * P], BF16, tag="hT")
                nc.scalar.activation(
                    out=hT[:, :W], in_=ph[:, :W], func=AF.Silu, bias=ub1_s[:, :1]
                )
                po = pp.tile([P, 4 * P], F32, tag="ph")
                nc.tensor.matmul(
                    out=po[:, :W], lhsT=uw2_s[:], rhs=hT[:, :W], start=True, stop=True
                )
                oT = sb.tile([P, 4 * P], F32, tag="oT")
                nc.scalar.activation(
                    out=oT[:, :W], in_=po[:, :W], func=AF.Identity, bias=ub2_s[:, :1]
                )
                nc.vector.tensor_tensor(
                    out=oT[:, :W], in0=oT[:, :W], in1=ownT[:, :W], op=OP.add
                )
                nc.sync.dma_start(
                    out=out[:, g * P : g * P + W], in_=oT[:, :W]
                )

    nc.compile()
    return nc


def _run(nc, in_maps, trace=False):
    return bass_utils.run_bass_kernel_spmd(
        nc, in_maps, core_ids=list(range(C)), trace=trace
    )


def make_in_maps(nodes, edge_index, edge_features, mw1, mb1, uw1, ub1, uw2, ub2,
                 ntiles_pc):
    N, D = nodes.shape
    NP_ = ntiles_pc * P
    N2 = NP_ * C
    kt, per_core = _host_prep(nodes, edge_index, edge_features, ntiles_pc)

    nodes_pad = np.zeros((N2, D), np.float32)
    nodes_pad[:N] = nodes
    iota4 = np.broadcast_to(
        np.tile(np.arange(P, dtype=np.float32), 4), (P, 4 * P)
    )
    ident = np.eye(P, dtype=np.float32)

    shared = dict(
        wt=_trunc_bf16(mw1[:D]),
        wb=_trunc_bf16(mw1[D:]),
        mbB=np.broadcast_to(
            np.tile(mb1.astype(np.float32), 4), (P, 4 * D)
        ).copy(),
        iota4=_trunc_bf16(np.ascontiguousarray(iota4)),
        ua=_trunc_bf16(uw1[:D]),
        ub=_trunc_bf16(uw1[D:]),
        uw2=_trunc_bf16(uw2),
        ub1c=np.ascontiguousarray(ub1.reshape(D, 1), np.float32),
        ub2c=np.ascontiguousarray(ub2.reshape(D, 1), np.float32),
        ident=ident,
    )
    in_maps = []
    for c in range(C):
        m = dict(shared)
        own = nodes_pad[c * NP_ : (c + 1) * NP_]  # [NP_, D]
        m["own_nodesT"] = np.ascontiguousarray(own.T)  # [P(d), NP_]
        m["nsefT"] = per_core[c]["nsefT"]
        m["dstoffT"] = per_core[c]["dstoffT"]
        in_maps.append(m)
    return kt, in_maps


def kernel(nodes, edge_index, edge_features, mw1, mb1, uw1, ub1, uw2, ub2):
    nodes = np.asarray(nodes, np.float32)
    edge_index = np.asarray(edge_index, np.int32)
    edge_features = np.asarray(edge_features, np.float32)
    N, D = nodes.shape
    ntiles_pc = math.ceil(N / (C * P))
    kt, in_maps = make_in_maps(
        nodes, edge_index, edge_features, mw1, mb1, uw1, ub1, uw2, ub2, ntiles_pc
    )
    nc = build_program(D, ntiles_pc, kt)
    res = _run(nc, in_maps)
    NP_ = ntiles_pc * P
    # out_own is [P(d? no: partition = n%128), ntiles*D] -> rows
    outs = [np.ascontiguousarray(res.results[c]["out_own"].T) for c in range(C)]
    out = np.concatenate(outs, axis=0)
    return out[:N].astype(np.float32)


if __name__ == "__main__":
    rng = np.random.default_rng(0)
    N, E, D = 4096, 16384, 128
    nodes = rng.standard_normal((N, D), dtype=np.float32)
    edge_index = rng.integers(0, N, (2, E)).astype(np.int32)
    ef = rng.standard_normal((E, D), dtype=np.float32)
    s2, s1 = 1 / np.sqrt(2 * D), 1 / np.sqrt(D)
    mw1 = rng.uniform(-s2, s2, (2 * D, D)).astype(np.float32)
    mb1 = rng.uniform(-s2, s2, D).astype(np.float32)
    uw1 = rng.uniform(-s2, s2, (2 * D, D)).astype(np.float32)
    ub1 = rng.uniform(-s2, s2, D).astype(np.float32)
    uw2 = rng.uniform(-s1, s1, (D, D)).astype(np.float32)
    ub2 = rng.uniform(-s1, s1, D).astype(np.float32)

    def silu(x):
        return x / (1 + np.exp(-x))

    def ref():
        src, dst = edge_index
        msg = silu(np.concatenate([nodes[src], ef], 1) @ mw1 + mb1)
        agg = np.zeros((N, D), np.float32)
        np.add.at(agg, dst, msg)
        upd = silu(np.concatenate([nodes, agg], 1) @ uw1 + ub1) @ uw2 + ub2
        return nodes + upd
    out = kernel(nodes, edge_index, ef, mw1, mb1, uw1, ub1, uw2, ub2)
    exp = ref()
    err = np.abs(out - exp).max() / np.abs(exp).max()
    print("tiny rel err:", err)


# revision 11
# speedup vs baseline: 1.8952x; 1.2131x over previous
"""Trainium2 Bass kernel for a GNN message-passing layer.

reference semantics (jax):
    src, dst = edge_index
    messages   = silu(concat(nodes[src], edge_features) @ mw1 + mb1)    # [E, D]
    aggregated = segment_sum(messages, dst, N)                          # [N, D]
    updated    = silu(concat(nodes, aggregated) @ uw1 + ub1) @ uw2 + ub2
    out        = nodes + updated

Distribution: destination-node partition across 8 cores. Each core owns a
contiguous 1/8 slice of the (padded) node range, aggregates exactly the
edges landing in its slice, and runs the update MLP on its slice. No
collectives.

Host-side work is limited to layout transforms of inputs (slicing,
padding, permutation/gather of input rows into slot order, per-tile
128x128 block transposes, bf16 byte-truncation, index tables) — no float
arithmetic.

Slot layout: edges are bucketed by destination node tile (128 dst nodes
per tile). Local tile t owns kt[t] edge tiles of 128 slots (kt = max
over cores, a compile-time constant); leftover slots are pads with
dst-offset -1 so their junk messages scatter with weight 0. The host
streams, per edge slot, BOTH the source-node row nodes[src] and the
edge-feature row (pre-transposed per 128-tile, bf16), so the device does
no gathers at all.

Device pipeline per core, per local node tile t:
  1. One contiguous DMA of the [ns^T | ef^T] chunk (bf16, [128, 2*kt*128]).
  2. Per 4-edge-tile chunk: per edge tile a 1-partition ones-matmul adds
     the message bias into PSUM (start=True), then ns/ef matmuls
     accumulate; one SiLU (PSUM -> SBUF bf16); one wide DVE is_equal
     builds all 4 one-hots at once (broadcast dst-offset columns vs a
     tiled iota).
  3. Per edge tile: a scatter matmul (lhsT=msg, rhs=one-hot)
     accumulating agg^T [d, j] in PSUM.
  Chunks are software-pipelined: chunk i's matmuls are emitted before
  chunk i-1's silu/scatter consumers so the PE stream never waits on the
  ACT/DVE roundtrip.
  4. Update MLP in transposed space (4 node tiles per group), residual,
     transpose back, store (partition-major output, host re-layouts).
"""

import math
import sys

sys.path.insert(0, "/opt/trn_rl_repo")

import numpy as np
import ml_dtypes

import concourse.bacc as bacc
import concourse.mybir as mybir
import concourse.tile as tile
from concourse import bass_utils

P = 128
C = 8  # cores
ONEHOT_GPSIMD = False  # Pool engine fails ISA check for is_equal tensor_tensor

F32 = mybir.dt.float32
BF16 = mybir.dt.bfloat16
AF = mybir.ActivationFunctionType
OP = mybir.AluOpType

NP_BF16 = ml_dtypes.bfloat16


def _trunc_bf16(a):
    """fp32 -> bf16 by byte truncation (pure byte slicing, no arithmetic)."""
    a = np.ascontiguousarray(a, np.float32)
    return a.view(np.uint16)[..., 1::2].view(NP_BF16)


def _blocksT(a):
    """[B*P, D] -> [P, B*D]: per-128-row-block transpose, blocks along free dim.

    out[d, b*D + e ... ] wait: out[x, b*P + e] = a[b*P + e, x]; requires D == P.
    """
    B = a.shape[0] // P
    D = a.shape[1]
    # [B, P, D] -> [B, D, P] -> [D?, ...] place block b at cols [b*P, (b+1)*P)
    t = a.reshape(B, P, D).transpose(2, 0, 1)  # [D, B, P]
    return np.ascontiguousarray(t.reshape(D, B * P))


def _host_prep(nodes, edge_index, edge_features, ntiles_pc):
    """Bucket edges by destination node tile; build per-core slot streams."""
    N, D = nodes.shape
    E = edge_index.shape[1]
    ntiles = ntiles_pc * C

    src = edge_index[0].astype(np.int64)
    dst = edge_index[1].astype(np.int64)
    tileid = dst // P
    order = np.argsort(tileid, kind="stable")
    ds = dst[order]
    ss = src[order]
    tid_s = tileid[order]

    counts = np.bincount(tileid, minlength=ntiles)
    cpt = counts.reshape(C, ntiles_pc)
    kt = [max(1, int(math.ceil(cpt[:, t].max() / P))) for t in range(ntiles_pc)]
    offs = np.zeros(ntiles_pc + 1, np.int64)
    np.cumsum(kt, out=offs[1:])
    sumkt = int(offs[-1])
    SL = sumkt * P  # slots per core

    tile_start = np.zeros(ntiles + 1, np.int64)
    np.cumsum(counts, out=tile_start[1:])
    rank = np.arange(E, dtype=np.int64) - tile_start[tid_s]
    core = tid_s // ntiles_pc
    t_local = tid_s % ntiles_pc
    slot = offs[t_local] * P + rank  # slot within the core's stream

    nodes16 = _trunc_bf16(nodes)
    ef16 = _trunc_bf16(edge_features)

    per_core = []
    for c in range(C):
        m = core == c
        sl_c = slot[m]
        # source rows + edge rows into slot order (pads stay zero)
        ns = np.zeros((SL, D), NP_BF16)
        ns[sl_c] = nodes16[ss[m]]
        ef = np.zeros((SL, D), NP_BF16)
        ef[sl_c] = ef16[order[m]]
        dof = np.full(SL, -1.0, np.float32)
        dof[sl_c] = (ds[m] - (ds[m] // P) * P).astype(np.float32)

        nsT = _blocksT(ns)  # [P, SL]
        efT = _blocksT(ef)  # [P, SL]
        # merged stream: per tile t, kt[t] ns-tiles then kt[t] ef-tiles
        nsef = np.empty((P, 2 * SL), NP_BF16)
        for t in range(ntiles_pc):
            a, b = int(offs[t]) * P, int(offs[t + 1]) * P
            w = b - a
            nsef[:, 2 * a : 2 * a + w] = nsT[:, a:b]
            nsef[:, 2 * a + w : 2 * b] = efT[:, a:b]
        dstoffT = np.ascontiguousarray(
            _trunc_bf16(dof.reshape(sumkt, P).T)
        )  # [P, sumkt]
        per_core.append(dict(nsefT=nsef, dstoffT=dstoffT))
    return kt, per_core


def build_program(D, ntiles_pc, kt, debug=False):
    """Build the SPMD Bass program (identical across cores)."""
    assert D == P
    nc = bacc.Bacc("TRN2", target_bir_lowering=False, debug=False, num_devices=C)
    NP_ = ntiles_pc * P
    offs = np.zeros(ntiles_pc + 1, np.int64)
    np.cumsum(kt, out=offs[1:])
    sumkt = int(offs[-1])
    ktmax = max(kt)

    d = lambda name, shape, dt=F32, kind="ExternalInput": nc.dram_tensor(
        name, shape, dt, kind=kind
    ).ap()

    nsef = d("nsefT", [P, 2 * sumkt * P], BF16)
    doff = d("dstoffT", [P, sumkt], BF16)
    ownT_d = d("own_nodesT", [P, NP_])
    wt = d("wt", [D, D], BF16)
    wb_ = d("wb", [D, D], BF16)
    mbB = d("mbB", [P, 8 * D])
    iotaB = d("iotaB", [P, ktmax * P], BF16)
    ua = d("ua", [D, D], BF16)
    ub = d("ub", [D, D], BF16)
    uw2 = d("uw2", [D, D], BF16)
    ub1c = d("ub1c", [P, 1])
    ub2c = d("ub2c", [P, 1])
    ident = d("ident", [P, P])
    out = d("out_own", [P, NP_], kind="ExternalOutput")
    aggdbg = d("aggdbg", [P, ntiles_pc * D], kind="ExternalOutput") if debug else None

    with tile.TileContext(nc) as tc:
        with (
            tc.tile_pool(name="const", bufs=1) as cp,
            tc.tile_pool(name="sb", bufs=3) as sb,
            tc.tile_pool(name="big", bufs=3) as bigp,
            tc.tile_pool(name="psum", bufs=2, space="PSUM") as pp,
            tc.tile_pool(name="psum1", bufs=1, space="PSUM") as pp1,
            tc.tile_pool(name="psum3", bufs=2, space="PSUM") as pp3,
        ):
            def load_const(ap, shape, dt=F32):
                t = cp.tile(shape, dt, tag=ap.name)
                nc.sync.dma_start(out=t[:], in_=ap[:])
                return t

            wt_s = load_const(wt, [D, D], BF16)
            wb_s = load_const(wb_, [D, D], BF16)
            mbB_s = load_const(mbB, [P, 8 * D])
            iotaB_s = load_const(iotaB, [P, ktmax * P], BF16)
            ua_s = load_const(ua, [D, D], BF16)
            ub_s = load_const(ub, [D, D], BF16)
            uw2_s = load_const(uw2, [D, D], BF16)
            ub1_s = load_const(ub1c, [P, 1])
            ub2_s = load_const(ub2c, [P, 1])
            id_s = load_const(ident, [P, P])
            doff_s = load_const(doff, [P, sumkt], BF16)
            aggT_all = cp.tile([P, ntiles_pc * D], F32, tag="aggT_all")

            # ---- stage 2: edge pipeline (software-pipelined by 1 chunk) ----
            CH = 8  # edge tiles per PSUM chunk (2 banks)
            chunks = []
            for t in range(ntiles_pc):
                nch = math.ceil(kt[t] / CH)
                for ci in range(nch):
                    chunks.append((t, ci, ci == 0, ci == nch - 1))

            state = {}  # t -> (chunk_tile, paggT, ohT)

            def produce(t, ci, first):
                KT = kt[t]
                if first:
                    W2 = 2 * KT * D
                    ctile = bigp.tile([P, 2 * ktmax * D], BF16, tag="chunk")
                    dma = nc.sync if (t % 2 == 0) else nc.scalar
                    dma.dma_start(
                        out=ctile[:, :W2],
                        in_=nsef[
                            :, 2 * int(offs[t]) * D : 2 * int(offs[t]) * D + W2
                        ],
                    )
                    paggT = pp1.tile([P, D], F32, tag="paggT")
                    ohT = bigp.tile([P, ktmax * P], BF16, tag="ohT")
                    a = int(offs[t])
                    nc.vector.tensor_tensor(
                        out=ohT[:, : KT * P].rearrange("p (f e) -> p f e", e=P),
                        in0=doff_s[:, a : a + KT].to_broadcast([P, KT, P]),
                        in1=iotaB_s[:, : KT * P].rearrange(
                            "p (f e) -> p f e", e=P
                        ),
                        op=OP.is_equal,
                    )
                    state[t] = (ctile, paggT, ohT)
                ctile, _, _ = state[t]
                k0 = ci * CH
                cw = min(CH, KT - k0)
                pmsg = pp3.tile([P, CH * P], F32, tag="pmsg")
                for j in range(cw):
                    k = k0 + j
                    reg = pmsg[:, j * P : (j + 1) * P]
                    nc.tensor.matmul(
                        out=reg, lhsT=ctile[:, k * D : (k + 1) * D], rhs=wt_s[:],
                        start=True, stop=False,
                    )
                    nc.tensor.matmul(
                        out=reg,
                        lhsT=ctile[:, (KT + k) * D : (KT + k + 1) * D],
                        rhs=wb_s[:],
                        start=False, stop=True,
                    )
                return (t, ci, cw, pmsg)

            def consume(pend, last):
                t, ci, cw, pmsg = pend
                KT = kt[t]
                k0 = ci * CH
                W = cw * P
                _, paggT, ohT = state[t]
                mpre = sb.tile([P, CH * P], BF16, tag="mpre")
                nc.vector.tensor_tensor(
                    out=mpre[:, :W], in0=pmsg[:, :W], in1=mbB_s[:, :W], op=OP.add
                )
                msg = sb.tile([P, CH * P], BF16, tag="msg")
                nc.scalar.activation(out=msg[:, :W], in_=mpre[:, :W], func=AF.Silu)
                for j in range(cw):
                    k = k0 + j
                    # aggT[d, j] += sum_e msg[e, d] * oh[e, j]
                    nc.tensor.matmul(
                        out=paggT[:],
                        lhsT=msg[:, j * P : (j + 1) * P],
                        rhs=ohT[:, k * P : (k + 1) * P],
                        start=(k == 0),
                        stop=(k == KT - 1),
                    )
                if last:
                    nc.vector.tensor_copy(
                        out=aggT_all[:, t * D : (t + 1) * D], in_=paggT[:]
                    )

            pend = None
            for t, ci, first, last in chunks:
                cur = produce(t, ci, first)
                if pend is not None:
                    consume(pend, pend_last)
                pend, pend_last = cur, last
            consume(pend, pend_last)
            if debug:
                nc.sync.dma_start(out=aggdbg[:], in_=aggT_all[:])

            # ---- stage 3: update MLP (transposed space, 4 node tiles/group;
            # output stays transposed [d, n] — host re-layouts) ----
            for g in range(0, ntiles_pc, 4):
                gw = min(4, ntiles_pc - g)
                W = gw * P
                ownT = sb.tile([P, 4 * P], F32, tag="ownT")
                nc.sync.dma_start(
                    out=ownT[:, :W], in_=ownT_d[:, g * P : g * P + W]
                )
                ownT16 = sb.tile([P, 4 * P], BF16, tag="ownT16")
                nc.vector.tensor_copy(out=ownT16[:, :W], in_=ownT[:, :W])
                agg16 = sb.tile([P, 4 * P], BF16, tag="agg16")
                nc.vector.tensor_copy(
                    out=agg16[:, :W], in_=aggT_all[:, g * D : g * D + W]
                )
                ph = pp.tile([P, 4 * P], F32, tag="ph")
                nc.tensor.matmul(
                    out=ph[:, :W], lhsT=ua_s[:], rhs=ownT16[:, :W],
                    start=True, stop=False,
                )
                nc.tensor.matmul(
                    out=ph[:, :W], lhsT=ub_s[:], rhs=agg16[:, :W],
                    start=False, stop=True,
                )
                hT = sb.tile([P, 4 * P], BF16, tag="hT")
                nc.scalar.activation(
                    out=hT[:, :W], in_=ph[:, :W], func=AF.Silu, bias=ub1_s[:, :1]
                )
                po = pp.tile([P, 4 * P], F32, tag="ph")
                nc.tensor.matmul(
                    out=po[:, :W], lhsT=uw2_s[:], rhs=hT[:, :W], start=True, stop=True
                )
                oT = sb.tile([P, 4 * P], F32, tag="oT")
                nc.scalar.activation(
                    out=oT[:, :W], in_=po[:, :W], func=AF.Identity, bias=ub2_s[:, :1]
                )
                nc.vector.tensor_tensor(
                    out=oT[:, :W], in0=oT[:, :W], in1=ownT[:, :W], op=OP.add
                )
                nc.sync.dma_start(
                    out=out[:, g * P : g * P + W], in_=oT[:, :W]
                )

    nc.compile()
    return nc


def _run(nc, in_maps, trace=False):
    return bass_utils.run_bass_kernel_spmd(
        nc, in_maps, core_ids=list(range(C)), trace=trace
    )


def make_in_maps(nodes, edge_index, edge_features, mw1, mb1, uw1, ub1, uw2, ub2,
                 ntiles_pc):
    N, D = nodes.shape
    NP_ = ntiles_pc * P
    N2 = NP_ * C
    kt, per_core = _host_prep(nodes, edge_index, edge_features, ntiles_pc)

    nodes_pad = np.zeros((N2, D), np.float32)
    nodes_pad[:N] = nodes
    ktmax = max(kt)
    iotaB = np.broadcast_to(
        np.tile(np.arange(P, dtype=np.float32), ktmax), (P, ktmax * P)
    )
    ident = np.eye(P, dtype=np.float32)

    shared = dict(
        wt=_trunc_bf16(mw1[:D]),
        wb=_trunc_bf16(mw1[D:]),
        mbB=np.broadcast_to(
            np.tile(mb1.astype(np.float32), 8), (P, 8 * D)
        ).copy(),
        iotaB=_trunc_bf16(np.ascontiguousarray(iotaB)),
        ua=_trunc_bf16(uw1[:D]),
        ub=_trunc_bf16(uw1[D:]),
        uw2=_trunc_bf16(uw2),
        ub1c=np.ascontiguousarray(ub1.reshape(D, 1), np.float32),
        ub2c=np.ascontiguousarray(ub2.reshape(D, 1), np.float32),
        ident=ident,
    )
    in_maps = []
    for c in range(C):
        m = dict(shared)
        own = nodes_pad[c * NP_ : (c + 1) * NP_]  # [NP_, D]
        m["own_nodesT"] = np.ascontiguousarray(own.T)  # [P(d), NP_]
        m["nsefT"] = per_core[c]["nsefT"]
        m["dstoffT"] = per_core[c]["dstoffT"]
        in_maps.append(m)
    return kt, in_maps


def kernel(nodes, edge_index, edge_features, mw1, mb1, uw1, ub1, uw2, ub2):
    nodes = np.asarray(nodes, np.float32)
    edge_index = np.asarray(edge_index, np.int32)
    edge_features = np.asarray(edge_features, np.float32)
    N, D = nodes.shape
    ntiles_pc = math.ceil(N / (C * P))
    kt, in_maps = make_in_maps(
        nodes, edge_index, edge_features, mw1, mb1, uw1, ub1, uw2, ub2, ntiles_pc
    )
    nc = build_program(D, ntiles_pc, kt)
    res = _run(nc, in_maps)
    NP_ = ntiles_pc * P
    # out_own is [P(d? no: partition = n%128), ntiles*D] -> rows
    outs = [np.ascontiguousarray(res.results[c]["out_own"].T) for c in range(C)]
    out = np.concatenate(outs, axis=0)
    return out[:N].astype(np.float32)


if __name__ == "__main__":
    rng = np.random.default_rng(0)
    N, E, D = 4096, 16384, 128
    nodes = rng.standard_normal((N, D), dtype=np.float32)
    edge_index = rng.integers(0, N, (2, E)).astype(np.int32)
    ef = rng.standard_normal((E, D), dtype=np.float32)
    s2, s1 = 1 / np.sqrt(2 * D), 1 / np.sqrt(D)
    mw1 = rng.uniform(-s2, s2, (2 * D, D)).astype(np.float32)
    mb1 = rng.uniform(-s2, s2, D).astype(np.float32)
    uw1 = rng.uniform(-s2, s2, (2 * D, D)).astype(np.float32)
    ub1 = rng.uniform(-s2, s2, D).astype(np.float32)
    uw2 = rng.uniform(-s1, s1, (D, D)).astype(np.float32)
    ub2 = rng.uniform(-s1, s1, D).astype(np.float32)

    def silu(x):
        return x / (1 + np.exp(-x))

    def ref():
        src, dst = edge_index
        msg = silu(np.concatenate([nodes[src], ef], 1) @ mw1 + mb1)
        agg = np.zeros((N, D), np.float32)
        np.add.at(agg, dst, msg)
        upd = silu(np.concatenate([nodes, agg], 1) @ uw1 + ub1) @ uw2 + ub2
        return nodes + upd
    out = kernel(nodes, edge_index, ef, mw1, mb1, uw1, ub1, uw2, ub2)
    exp = ref()
    err = np.abs(out - exp).max() / np.abs(exp).max()
    print("tiny rel err:", err)
